# revision 29
# baseline (speedup 1.0000x reference)
"""AlignmentBlock kernel for 8 TRN2 NeuronCores (data-parallel over B).

Math (per batch b, one core per batch):
  s_hat[s,a] = (LN(signal[s]) * g1 + b1) @ sig_W.T, zeroed where signal_mask
  b_hat[t,a] = (LN(bases[t]) * g2 + b2) @ bases_W.T
  out[t,s,k] = aln[t,s,k] + gelu( sum_a b_hat[t,a]*s_hat[s,a]*out_W[k,a] + out_b[k] )

Key tricks (measured 148.5us -> ~81us on HW):
- The [B,T,S,A] intermediate is never materialized: for each t the
  projection collapses to s_hat @ (out_W.T * b_hat[t]) — a [65,128]^T x
  [65, 8t*64] matmul per 128-column chunk of s_hat (row 65 of the
  stationary is ones, row 65 of the moving operand is out_b).
- bf16 aln I/O: the host packs aln[b] to bf16 in the exact SBUF tile
  layout and unpacks the bf16 result to fp32 after the run.  Halves HBM
  traffic (2 x 12.6MB instead of 2 x 25.2MB per core); every aln DMA is
  a fully contiguous 1-2MB transfer.  The layout is c-outer
  (aln_p[tg, p, c, g*64+k]) so every epilogue op hits contiguous
  512-element runs per partition.
- Mask sort: the s axis is freely permutable (the host controls the
  signal row order, the aln packing and the unpacking), so each core's
  s positions are sorted unmasked-first.  For a fully-masked 128-column
  chunk the projection input is exactly out_b, so out = aln +
  gelu(out_b) — one broadcast bf16 add, no matmul / no gelu.  Only
  eu = max_b ceil(unmasked_b/128) chunks (5 of 8 for ~50% masks) take
  the matmul path; the kernel graph is built per call with that eu.
- Queue split: aln loads own the sync HWDGE ring (params first, they're
  small), stores own the scalar HWDGE ring; with both streams the kernel
  sits at the shared HBM-stack roofline (~26MB/core at ~370GB/s).
- Epilogue fused pairwise: two matmuls fill a 2-bank PSUM tile, then ONE
  gelu (ScalarE, f32 -> bf16) and ONE contiguous bf16 add (VectorE, 2x
  rate) cover both banks.  wfull (out_W.T * b_hat) is built per group
  just-in-time on VectorE so it hides in epilogue slack instead of
  delaying the signal-branch preamble.  GpSimd runs no bulk tensor ops —
  its SBUF traffic stalls concurrent DVE work 3-6x.

Matmul operands are bf16; accumulation stays fp32 in PSUM. LayerNorm
affine and all parameter reshapes/transposes are folded on the host.
"""

import numpy as np
import ml_dtypes

import concourse.bass as bass
import concourse.tile as tile
from concourse import bacc, mybir
from concourse.bass_utils import run_bass_kernel_spmd
from concourse.masks import make_identity

F32 = mybir.dt.float32
BF16 = mybir.dt.bfloat16
AF = mybir.ActivationFunctionType
ALU = mybir.AluOpType

B, T, S, E, A = 8, 96, 1024, 256, 64
LN_EPS = 1e-5
GT = 8           # t's per packed aln block (1MB bf16)
TG = T // GT     # 12 blocks
NJ = S // 128    # 8 s-chunks of 128
BF = ml_dtypes.bfloat16

# main-loop groups: (block index, #blocks fused into one az tile);
# small groups first (early first store) and last (short drain tail)
GROUPS = [(0, 1), (1, 1), (2, 2), (4, 2), (6, 2), (8, 2), (10, 1), (11, 1)]


def build_nc(eu=NJ):
    nc = bacc.Bacc(target_bir_lowering=False)

    sig = nc.declare_dram_parameter("signal", [128, eu, E], F32, isOutput=False)
    bas = nc.declare_dram_parameter("bases", [T, E], F32, isOutput=False)
    aln = nc.declare_dram_parameter("aln", [TG, 128, NJ, GT * A], BF16,
                                    isOutput=False)
    mskT = nc.declare_dram_parameter("maskT", [128, eu], F32, isOutput=False)
    mrow = nc.declare_dram_parameter("mrow", [1, eu * 128], BF16, isOutput=False)
    A1 = nc.declare_dram_parameter("A1", [E, A], BF16, isOutput=False)
    c1 = nc.declare_dram_parameter("c1", [1, A], BF16, isOutput=False)
    A2 = nc.declare_dram_parameter("A2", [E, A], BF16, isOutput=False)
    c2 = nc.declare_dram_parameter("c2", [1, A], BF16, isOutput=False)
    WtT = nc.declare_dram_parameter("WtT", [A, A], F32, isOutput=False)
    outbrep = nc.declare_dram_parameter("outbrep", [1, T * A], BF16,
                                        isOutput=False)
    out = nc.declare_dram_parameter("out", [TG, 128, NJ, GT * A], BF16,
                                    isOutput=True)

    aln2 = aln.ap().rearrange("(u v) p c x -> u p v c x", v=2)
    out2 = out.ap().rearrange("(u v) p c x -> u p v c x", v=2)

    with tile.TileContext(nc) as tc:
        with (
            tc.tile_pool(name="singles", bufs=1) as singles,
            tc.tile_pool(name="alnp2", bufs=4) as alnp2,
            tc.tile_pool(name="alnp1", bufs=4) as alnp1,
        ):
            # ---------- constants / params (sync ring, ahead of aln) ----------
            ident = singles.tile([128, 128], F32)
            make_identity(nc, ident)
            eps_t = singles.tile([128, 1], F32)
            nc.vector.memset(eps_t, LN_EPS)
            ones_row = singles.tile([1, 128], BF16)
            nc.vector.memset(ones_row, 1.0)

            bas_sb = singles.tile([T, E], F32)
            nc.sync.dma_start(out=bas_sb, in_=bas.ap())
            m_sb = singles.tile([128, eu], F32)
            nc.sync.dma_start(out=m_sb, in_=mskT.ap())
            m_row = singles.tile([1, eu * 128], BF16)
            nc.sync.dma_start(out=m_row, in_=mrow.ap())
            A1_sb = singles.tile([128, 2, A], BF16)
            nc.sync.dma_start(
                out=A1_sb, in_=A1.ap().rearrange("(h e) a -> e h a", e=128)
            )
            A2_sb = singles.tile([128, 2, A], BF16)
            nc.sync.dma_start(
                out=A2_sb, in_=A2.ap().rearrange("(h e) a -> e h a", e=128)
            )
            c1_sb = singles.tile([1, A], BF16)
            nc.sync.dma_start(out=c1_sb, in_=c1.ap())
            c2_sb = singles.tile([1, A], BF16)
            nc.sync.dma_start(out=c2_sb, in_=c2.ap())
            WtT_sb = singles.tile([A, A], F32)
            nc.sync.dma_start(out=WtT_sb, in_=WtT.ap())
            wfull = singles.tile([A + 1, T, A], BF16)
            nc.sync.dma_start(
                out=wfull[A:A + 1, :, :],
                in_=outbrep.ap().rearrange("x (t k) -> x t k", t=T),
            )
            sig_sb = singles.tile([128, eu, E], F32)
            nc.sync.dma_start(out=sig_sb, in_=sig.ap())

            with (
                tc.tile_pool(name="pre", bufs=2) as pre,
                tc.tile_pool(name="psum_pre", bufs=2, space="PSUM") as psum_pre,
            ):
                # ---------- bases branch: LN + project -> bhT [64, 96] ----------
                bst = pre.tile([T, 6], F32, tag="pp_small")
                nc.vector.bn_stats(bst, bas_sb)
                bmv = pre.tile([T, 2], F32, tag="pp_small")
                nc.vector.bn_aggr(bmv, bst)
                brs = pre.tile([T, 1], F32, tag="pp_small")
                nc.scalar.activation(brs, bmv[:, 1:2], AF.Sqrt, bias=eps_t[0:T])
                nc.vector.reciprocal(brs, brs)
                xb = pre.tile([T, E], F32)
                nc.vector.tensor_scalar(
                    out=xb, in0=bas_sb, scalar1=bmv[:, 0:1], scalar2=brs,
                    op0=ALU.subtract, op1=ALU.mult,
                )
                xbT = pre.tile([128, 2, T], BF16)
                for h in range(2):
                    ptr = psum_pre.tile([128, T], F32, tag="pp_psum")
                    nc.tensor.transpose(
                        ptr, xb[:, h * 128:(h + 1) * 128], ident[0:T, 0:T]
                    )
                    nc.scalar.copy(xbT[:, h, :], ptr)
                bh_ps = psum_pre.tile([A, T], F32, tag="pp_psum")
                nc.tensor.matmul(bh_ps, A2_sb[:, 0, :], xbT[:, 0, :],
                                 start=True, stop=False)
                nc.tensor.matmul(bh_ps, A2_sb[:, 1, :], xbT[:, 1, :],
                                 start=False, stop=False)
                nc.tensor.matmul(bh_ps, c2_sb, ones_row[:, 0:T],
                                 start=False, stop=True)
                bhT = singles.tile([A, T], F32)
                nc.vector.tensor_copy(bhT, bh_ps)

                # ---------- signal branch: LN (masked) + transpose ----------
                xnT = singles.tile([128, 2, eu * 128], BF16)
                st8 = pre.tile([128, eu, 6], F32, tag="pp_small")
                for j in range(eu):
                    nc.vector.bn_stats(st8[:, j, :], sig_sb[:, j, :])
                mv8 = pre.tile([128, eu, 2], F32, tag="pp_small")
                for j in range(eu):
                    nc.vector.bn_aggr(mv8[:, j, :], st8[:, j, :])
                rsm8 = pre.tile([128, eu], F32, tag="pp_small")
                nc.scalar.activation(rsm8, mv8[:, :, 1], AF.Sqrt, bias=eps_t)
                nc.vector.reciprocal(rsm8, rsm8)
                nc.vector.tensor_mul(rsm8, rsm8, m_sb)

                # gelu(out_b) broadcast tile for masked chunks
                if eu < NJ:
                    g1 = singles.tile([1, A], BF16)
                    nc.scalar.activation(g1, wfull[A:A + 1, 0, :], AF.Gelu)
                    gps = psum_pre.tile([128, A], F32, tag="pp_psum")
                    nc.tensor.matmul(gps, ones_row, g1, start=True, stop=True)
                    gcbr = singles.tile([128, GT, A], BF16)
                    for i in range(GT):
                        nc.scalar.copy(gcbr[:, i, :], gps)

                for j in range(eu):
                    xn = pre.tile([128, E], F32)
                    nc.vector.tensor_scalar(
                        out=xn, in0=sig_sb[:, j, :],
                        scalar1=mv8[:, j, 0:1], scalar2=rsm8[:, j:j + 1],
                        op0=ALU.subtract, op1=ALU.mult,
                    )
                    ptr = psum_pre.tile([128, 256], F32, tag="pp_psum")
                    for h in range(2):
                        nc.tensor.transpose(
                            ptr[:, h * 128:(h + 1) * 128],
                            xn[:, h * 128:(h + 1) * 128], ident)
                    nc.scalar.copy(
                        xnT[:, :, j * 128:(j + 1) * 128],
                        ptr.rearrange("p (h q) -> p h q", h=2))

                # ---------- per-t weights wfull [65, T, 64] bf16 ----------
                TQ = T // 4
                for q in range(4):
                    WtT_bc = bass.AP(
                        tensor=WtT_sb.tensor, offset=WtT_sb.offset,
                        ap=[WtT_sb.ap[0], [0, TQ], WtT_sb.ap[1]],
                    )
                    bq = bhT[:, q * TQ:(q + 1) * TQ]
                    bhT_bc = bass.AP(
                        tensor=bq.tensor, offset=bq.offset,
                        ap=[bq.ap[0], bq.ap[1], [0, A]],
                    )
                    nc.vector.tensor_tensor(
                        wfull[0:A, q * TQ:(q + 1) * TQ, :], WtT_bc, bhT_bc,
                        ALU.mult)

                # ---------- project signal -> shp [65, eu*128] bf16 ----------
                # column u = C*128 + p  <->  sorted s position ; row 64 = ones
                shp = singles.tile([A + 1, eu * 128], BF16)
                for j in range(eu):
                    cs = slice(j * 128, (j + 1) * 128)
                    pp = psum_pre.tile([A, 128], F32, tag="pp_psum")
                    nc.tensor.matmul(
                        pp, A1_sb[:, 0, :], xnT[:, 0, cs],
                        start=True, stop=False)
                    nc.tensor.matmul(
                        pp, A1_sb[:, 1, :], xnT[:, 1, cs],
                        start=False, stop=False)
                    nc.tensor.matmul(
                        pp, c1_sb, m_row[:, cs],
                        start=False, stop=True)
                    nc.scalar.copy(shp[0:A, cs], pp)
                nc.vector.memset(shp[A:A + 1, :], 1.0)

            # ---------- main loop ----------
            with (
                tc.tile_pool(name="zp", bufs=8) as zp,
                tc.tile_pool(name="psum_main", bufs=4, space="PSUM") as psum_main,
            ):
                for blk, nb in GROUPS:
                    if nb == 2:
                        az = alnp2.tile([128, 2, NJ, GT * A], BF16)
                        nc.sync.dma_start(out=az, in_=aln2[blk // 2])
                    else:
                        az = alnp1.tile([128, 1, NJ, GT * A], BF16)
                        nc.sync.dma_start(
                            out=az[:, 0, :, :], in_=aln.ap()[blk])
                    for c in range(eu):
                        ps = psum_main.tile([128, 2, GT, A], F32)
                        for h in range(nb):
                            t0 = (blk + h) * GT
                            nc.tensor.matmul(
                                ps[:, h, :, :],
                                shp[:, c * 128:(c + 1) * 128],
                                wfull[:, t0:t0 + GT, :],
                                start=True, stop=True,
                            )
                        zc = zp.tile([128, 2, GT, A], BF16)
                        nc.scalar.activation(
                            zc[:, 0:nb, :, :], ps[:, 0:nb, :, :], AF.Gelu)
                        zcv = zc.rearrange("p v g k -> p v (g k)")
                        sl = az[:, 0:nb, c, :]
                        nc.vector.tensor_add(sl, sl, zcv[:, 0:nb, :])
                    if eu < NJ:
                        slm = az[:, 0:nb, eu:NJ, :]
                        gb = bass.AP(
                            tensor=gcbr.tensor, offset=gcbr.offset,
                            ap=[gcbr.ap[0], [0, nb], [0, NJ - eu], [1, GT * A]],
                        )
                        nc.vector.tensor_tensor(slm, slm, gb, ALU.add)
                    if nb == 2:
                        nc.scalar.dma_start(out=out2[blk // 2], in_=az)
                    else:
                        nc.scalar.dma_start(
                            out=out.ap()[blk], in_=az[:, 0, :, :])

    nc.finalize()
    return nc


def _perm_for_mask(mask_b):
    # stable sort, unmasked (False) first
    return np.argsort(mask_b, kind="stable")


def _pack_aln(aln_b, perm):
    # aln_p[tg, p, c, g*64+k] = aln[tg*GT+g, perm[c*128+p], k], bf16
    a16 = np.asarray(aln_b, np.float32).astype(BF)[:, perm, :]
    a16 = a16.reshape(TG, GT, NJ, 128, A).transpose(0, 3, 2, 1, 4)
    return np.ascontiguousarray(a16).reshape(TG, 128, NJ, GT * A)


def _unpack_out(o, perm):
    # inverse of _pack_aln, bf16 -> fp32 [T, S, A]
    o = np.asarray(o).reshape(TG, 128, NJ, GT, A).transpose(0, 3, 2, 1, 4)
    o = np.ascontiguousarray(o).reshape(T, S, A).astype(np.float32)
    res = np.empty_like(o)
    res[:, perm, :] = o
    return res


def _prep_in_maps(signal, bases, aln, signal_mask,
                  sig_norm_g, sig_norm_b, bases_norm_g, bases_norm_b,
                  sig_W, bases_W, out_W, out_b):
    signal = np.asarray(signal, np.float32)
    bases = np.asarray(bases, np.float32)
    aln = np.asarray(aln, np.float32)
    mask = np.asarray(signal_mask)
    A1 = np.ascontiguousarray(
        (np.asarray(sig_W, np.float32) * np.asarray(sig_norm_g, np.float32)).T
    ).astype(BF)
    c1 = (np.asarray(sig_W, np.float32) @ np.asarray(sig_norm_b, np.float32))[
        None].astype(BF)
    A2 = np.ascontiguousarray(
        (np.asarray(bases_W, np.float32) * np.asarray(bases_norm_g, np.float32)).T
    ).astype(BF)
    c2 = (np.asarray(bases_W, np.float32) @ np.asarray(bases_norm_b, np.float32))[
        None].astype(BF)
    WtT = np.ascontiguousarray(np.asarray(out_W, np.float32).T)
    outbrep = np.ascontiguousarray(
        np.tile(np.asarray(out_b, np.float32), T)[None]
    ).astype(BF)

    perms = [_perm_for_mask(mask[b]) for b in range(B)]
    eu = max(
        int(np.ceil((~mask[b]).sum() / 128.0)) for b in range(B))
    eu = max(eu, 1)

    in_maps = []
    for b in range(B):
        perm = perms[b]
        mf_s = (1.0 - mask[b].astype(np.float32))[perm]  # sorted; 1 = unmasked
        sig_dev = signal[b][perm[:128 * eu]].reshape(eu, 128, E)
        sig_dev = np.ascontiguousarray(sig_dev.transpose(1, 0, 2))
        in_maps.append({
            "signal": sig_dev,
            "bases": np.ascontiguousarray(bases[b]),
            "aln": _pack_aln(aln[b], perm),
            "maskT": np.ascontiguousarray(
                mf_s[:128 * eu].reshape(eu, 128).T),
            "mrow": np.ascontiguousarray(
                mf_s[:128 * eu][None]).astype(BF),
            "A1": A1, "c1": np.ascontiguousarray(c1),
            "A2": A2, "c2": np.ascontiguousarray(c2),
            "WtT": WtT, "outbrep": outbrep,
        })
    return in_maps, perms, eu


def _gather_out(res, perms):
    return np.stack(
        [_unpack_out(res.results[i]["out"], perms[i]) for i in range(B)],
        axis=0)


def _run(inputs, **kw):
    in_maps, perms, eu = _prep_in_maps(**inputs)
    nc = build_nc(eu)
    res = run_bass_kernel_spmd(nc, in_maps, core_ids=list(range(B)), **kw)
    return _gather_out(res, perms), res


def kernel(**inputs) -> np.ndarray:
    out, _ = _run(inputs)
    return out


# revision 30
# speedup vs baseline: 1.0188x; 1.0188x over previous
"""AlignmentBlock kernel for 8 TRN2 NeuronCores (data-parallel over B).

Math (per batch b, one core per batch):
  s_hat[s,a] = (LN(signal[s]) * g1 + b1) @ sig_W.T, zeroed where signal_mask
  b_hat[t,a] = (LN(bases[t]) * g2 + b2) @ bases_W.T
  out[t,s,k] = aln[t,s,k] + gelu( sum_a b_hat[t,a]*s_hat[s,a]*out_W[k,a] + out_b[k] )

Key tricks (measured 148.5us -> ~81us on HW):
- The [B,T,S,A] intermediate is never materialized: for each t the
  projection collapses to s_hat @ (out_W.T * b_hat[t]) — a [65,128]^T x
  [65, 8t*64] matmul per 128-column chunk of s_hat (row 65 of the
  stationary is ones, row 65 of the moving operand is out_b).
- bf16 aln I/O: the host packs aln[b] to bf16 in the exact SBUF tile
  layout and unpacks the bf16 result to fp32 after the run.  Halves HBM
  traffic (2 x 12.6MB instead of 2 x 25.2MB per core); every aln DMA is
  a fully contiguous 1-2MB transfer.  The layout is c-outer
  (aln_p[tg, p, c, g*64+k]) so every epilogue op hits contiguous
  512-element runs per partition.
- Mask sort: the s axis is freely permutable (the host controls the
  signal row order, the aln packing and the unpacking), so each core's
  s positions are sorted unmasked-first.  For a fully-masked 128-column
  chunk the projection input is exactly out_b, so out = aln +
  gelu(out_b) — one broadcast bf16 add, no matmul / no gelu.  Only
  eu = max_b ceil(unmasked_b/128) chunks (5 of 8 for ~50% masks) take
  the matmul path; the kernel graph is built per call with that eu.
- Queue split: aln loads own the sync HWDGE ring (params first, they're
  small), stores own the scalar HWDGE ring; with both streams the kernel
  sits at the shared HBM-stack roofline (~26MB/core at ~370GB/s).
- Epilogue fused pairwise: two matmuls fill a 2-bank PSUM tile, then ONE
  gelu (ScalarE, f32 -> bf16) and ONE contiguous bf16 add (VectorE, 2x
  rate) cover both banks.  wfull (out_W.T * b_hat) is built per group
  just-in-time on VectorE so it hides in epilogue slack instead of
  delaying the signal-branch preamble.  GpSimd runs no bulk tensor ops —
  its SBUF traffic stalls concurrent DVE work 3-6x.

Matmul operands are bf16; accumulation stays fp32 in PSUM. LayerNorm
affine and all parameter reshapes/transposes are folded on the host.
"""

import numpy as np
import ml_dtypes

import concourse.bass as bass
import concourse.tile as tile
from concourse import bacc, mybir
from concourse.bass_utils import run_bass_kernel_spmd
from concourse.masks import make_identity

F32 = mybir.dt.float32
BF16 = mybir.dt.bfloat16
AF = mybir.ActivationFunctionType
ALU = mybir.AluOpType

B, T, S, E, A = 8, 96, 1024, 256, 64
LN_EPS = 1e-5
GT = 8           # t's per packed aln block (1MB bf16)
TG = T // GT     # 12 blocks
NJ = S // 128    # 8 s-chunks of 128
BF = ml_dtypes.bfloat16

# main-loop groups: (block index, #blocks fused into one az tile);
# small groups first (early first store) and last (short drain tail)
GROUPS = [(0, 1), (1, 1), (2, 2), (4, 2), (6, 2), (8, 2), (10, 1), (11, 1)]


def build_nc(eu=NJ):
    nc = bacc.Bacc(target_bir_lowering=False)

    sig = nc.declare_dram_parameter("signal", [128, eu, E], F32, isOutput=False)
    bas = nc.declare_dram_parameter("bases", [T, E], F32, isOutput=False)
    aln = nc.declare_dram_parameter("aln", [TG, 128, NJ, GT * A], BF16,
                                    isOutput=False)
    mskT = nc.declare_dram_parameter("maskT", [128, eu], F32, isOutput=False)
    mrow = nc.declare_dram_parameter("mrow", [1, eu * 128], BF16, isOutput=False)
    A1 = nc.declare_dram_parameter("A1", [E, A], BF16, isOutput=False)
    c1 = nc.declare_dram_parameter("c1", [1, A], BF16, isOutput=False)
    A2 = nc.declare_dram_parameter("A2", [E, A], BF16, isOutput=False)
    c2 = nc.declare_dram_parameter("c2", [1, A], BF16, isOutput=False)
    WtT = nc.declare_dram_parameter("WtT", [A, A], F32, isOutput=False)
    outbrep = nc.declare_dram_parameter("outbrep", [1, T * A], BF16,
                                        isOutput=False)
    out = nc.declare_dram_parameter("out", [TG, 128, NJ, GT * A], BF16,
                                    isOutput=True)

    aln2 = aln.ap().rearrange("(u v) p c x -> u p v c x", v=2)
    out2 = out.ap().rearrange("(u v) p c x -> u p v c x", v=2)

    with tile.TileContext(nc) as tc:
        with (
            tc.tile_pool(name="singles", bufs=1) as singles,
            tc.tile_pool(name="alnp2", bufs=4) as alnp2,
            tc.tile_pool(name="alnp1", bufs=4) as alnp1,
        ):
            # ---------- constants / params (sync ring, ahead of aln) ----------
            ident = singles.tile([128, 128], F32)
            make_identity(nc, ident)
            eps_t = singles.tile([128, 1], F32)
            nc.vector.memset(eps_t, LN_EPS)
            ones_row = singles.tile([1, 128], BF16)
            nc.vector.memset(ones_row, 1.0)

            bas_sb = singles.tile([T, E], F32)
            nc.sync.dma_start(out=bas_sb, in_=bas.ap())
            m_sb = singles.tile([128, eu], F32)
            nc.sync.dma_start(out=m_sb, in_=mskT.ap())
            m_row = singles.tile([1, eu * 128], BF16)
            nc.sync.dma_start(out=m_row, in_=mrow.ap())
            A1_sb = singles.tile([128, 2, A], BF16)
            nc.sync.dma_start(
                out=A1_sb, in_=A1.ap().rearrange("(h e) a -> e h a", e=128)
            )
            A2_sb = singles.tile([128, 2, A], BF16)
            nc.sync.dma_start(
                out=A2_sb, in_=A2.ap().rearrange("(h e) a -> e h a", e=128)
            )
            c1_sb = singles.tile([1, A], BF16)
            nc.sync.dma_start(out=c1_sb, in_=c1.ap())
            c2_sb = singles.tile([1, A], BF16)
            nc.sync.dma_start(out=c2_sb, in_=c2.ap())
            WtT_sb = singles.tile([A, A], F32)
            nc.sync.dma_start(out=WtT_sb, in_=WtT.ap())
            wfull = singles.tile([A + 1, T, A], BF16)
            nc.sync.dma_start(
                out=wfull[A:A + 1, :, :],
                in_=outbrep.ap().rearrange("x (t k) -> x t k", t=T),
            )
            sig_sb = singles.tile([128, eu, E], F32)
            nc.sync.dma_start(out=sig_sb, in_=sig.ap())

            with (
                tc.tile_pool(name="pre", bufs=2) as pre,
                tc.tile_pool(name="psum_pre", bufs=2, space="PSUM") as psum_pre,
            ):
                # ---------- bases branch: LN + project -> bhT [64, 96] ----------
                bst = pre.tile([T, 6], F32, tag="pp_small")
                nc.vector.bn_stats(bst, bas_sb)
                bmv = pre.tile([T, 2], F32, tag="pp_small")
                nc.vector.bn_aggr(bmv, bst)
                brs = pre.tile([T, 1], F32, tag="pp_small")
                nc.scalar.activation(brs, bmv[:, 1:2], AF.Sqrt, bias=eps_t[0:T])
                nc.vector.reciprocal(brs, brs)
                xb = pre.tile([T, E], F32)
                nc.vector.tensor_scalar(
                    out=xb, in0=bas_sb, scalar1=bmv[:, 0:1], scalar2=brs,
                    op0=ALU.subtract, op1=ALU.mult,
                )
                xbT = pre.tile([128, 2, T], BF16)
                for h in range(2):
                    ptr = psum_pre.tile([128, T], F32, tag="pp_psum")
                    nc.tensor.transpose(
                        ptr, xb[:, h * 128:(h + 1) * 128], ident[0:T, 0:T]
                    )
                    nc.scalar.copy(xbT[:, h, :], ptr)
                bh_ps = psum_pre.tile([A, T], F32, tag="pp_psum")
                nc.tensor.matmul(bh_ps, A2_sb[:, 0, :], xbT[:, 0, :],
                                 start=True, stop=False)
                nc.tensor.matmul(bh_ps, A2_sb[:, 1, :], xbT[:, 1, :],
                                 start=False, stop=False)
                nc.tensor.matmul(bh_ps, c2_sb, ones_row[:, 0:T],
                                 start=False, stop=True)
                bhT = singles.tile([A, T], F32)
                nc.vector.tensor_copy(bhT, bh_ps)

                # ---------- signal branch: LN (masked) + transpose ----------
                xnT = singles.tile([128, 2, eu * 128], BF16)
                st8 = pre.tile([128, eu, 6], F32, tag="pp_small")
                for j in range(eu):
                    nc.vector.bn_stats(st8[:, j, :], sig_sb[:, j, :])
                mv8 = pre.tile([128, eu, 2], F32, tag="pp_small")
                for j in range(eu):
                    nc.vector.bn_aggr(mv8[:, j, :], st8[:, j, :])
                rsm8 = pre.tile([128, eu], F32, tag="pp_small")
                nc.scalar.activation(rsm8, mv8[:, :, 1], AF.Sqrt, bias=eps_t)
                nc.vector.reciprocal(rsm8, rsm8)
                nc.vector.tensor_mul(rsm8, rsm8, m_sb)
                for j in range(eu):
                    xn = pre.tile([128, E], F32)
                    nc.vector.tensor_scalar(
                        out=xn, in0=sig_sb[:, j, :],
                        scalar1=mv8[:, j, 0:1], scalar2=rsm8[:, j:j + 1],
                        op0=ALU.subtract, op1=ALU.mult,
                    )
                    ptr = psum_pre.tile([128, 256], F32, tag="pp_psum")
                    for h in range(2):
                        nc.tensor.transpose(
                            ptr[:, h * 128:(h + 1) * 128],
                            xn[:, h * 128:(h + 1) * 128], ident)
                    nc.scalar.copy(
                        xnT[:, :, j * 128:(j + 1) * 128],
                        ptr.rearrange("p (h q) -> p h q", h=2))

                # ---------- project signal -> shp [65, eu*128] bf16 ----------
                # column u = C*128 + p  <->  sorted s position ; row 64 = ones
                shp = singles.tile([A + 1, eu * 128], BF16)
                for j in range(eu):
                    cs = slice(j * 128, (j + 1) * 128)
                    pp = psum_pre.tile([A, 128], F32, tag="pp_psum")
                    nc.tensor.matmul(
                        pp, A1_sb[:, 0, :], xnT[:, 0, cs],
                        start=True, stop=False)
                    nc.tensor.matmul(
                        pp, A1_sb[:, 1, :], xnT[:, 1, cs],
                        start=False, stop=False)
                    nc.tensor.matmul(
                        pp, c1_sb, m_row[:, cs],
                        start=False, stop=True)
                    nc.scalar.copy(shp[0:A, cs], pp)
                nc.vector.memset(shp[A:A + 1, :], 1.0)

                # ---------- gelu(out_b) broadcast tile for masked chunks -------
                # (emitted after the shp chain so its ACT ops don't delay shp;
                # only the cheap adds consume it)
                if eu < NJ:
                    g1 = singles.tile([1, A], BF16)
                    nc.scalar.activation(g1, wfull[A:A + 1, 0, :], AF.Gelu)
                    gps = psum_pre.tile([128, A], F32, tag="pp_psum")
                    nc.tensor.matmul(gps, ones_row, g1, start=True, stop=True)
                    gcbr = singles.tile([128, GT, A], BF16)
                    for i in range(GT):
                        nc.scalar.copy(gcbr[:, i, :], gps)

            # ---------- main loop ----------
            with (
                tc.tile_pool(name="zp", bufs=8) as zp,
                tc.tile_pool(name="psum_main", bufs=4, space="PSUM") as psum_main,
            ):
                for blk, nb in GROUPS:
                    if nb == 2:
                        az = alnp2.tile([128, 2, NJ, GT * A], BF16)
                        nc.sync.dma_start(out=az, in_=aln2[blk // 2])
                    else:
                        az = alnp1.tile([128, 1, NJ, GT * A], BF16)
                        nc.sync.dma_start(
                            out=az[:, 0, :, :], in_=aln.ap()[blk])
                    # wfull piece for this group's t-range (rows 0..63:
                    # out_W.T * b_hat[t]); built just-in-time on VectorE so
                    # it hides in the epilogue slack instead of delaying shp
                    tw = nb * GT
                    t0w = blk * GT
                    WtT_bc = bass.AP(
                        tensor=WtT_sb.tensor, offset=WtT_sb.offset,
                        ap=[WtT_sb.ap[0], [0, tw], WtT_sb.ap[1]],
                    )
                    bq = bhT[:, t0w:t0w + tw]
                    bhT_bc = bass.AP(
                        tensor=bq.tensor, offset=bq.offset,
                        ap=[bq.ap[0], bq.ap[1], [0, A]],
                    )
                    nc.vector.tensor_tensor(
                        wfull[0:A, t0w:t0w + tw, :], WtT_bc, bhT_bc, ALU.mult)
                    for c in range(eu):
                        ps = psum_main.tile([128, 2, GT, A], F32)
                        for h in range(nb):
                            t0 = (blk + h) * GT
                            nc.tensor.matmul(
                                ps[:, h, :, :],
                                shp[:, c * 128:(c + 1) * 128],
                                wfull[:, t0:t0 + GT, :],
                                start=True, stop=True,
                            )
                        zc = zp.tile([128, 2, GT, A], BF16)
                        nc.scalar.activation(
                            zc[:, 0:nb, :, :], ps[:, 0:nb, :, :], AF.Gelu)
                        zcv = zc.rearrange("p v g k -> p v (g k)")
                        sl = az[:, 0:nb, c, :]
                        nc.vector.tensor_add(sl, sl, zcv[:, 0:nb, :])
                    if eu < NJ:
                        slm = az[:, 0:nb, eu:NJ, :]
                        gb = bass.AP(
                            tensor=gcbr.tensor, offset=gcbr.offset,
                            ap=[gcbr.ap[0], [0, nb], [0, NJ - eu], [1, GT * A]],
                        )
                        nc.vector.tensor_tensor(slm, slm, gb, ALU.add)
                    if nb == 2:
                        nc.scalar.dma_start(out=out2[blk // 2], in_=az)
                    else:
                        nc.scalar.dma_start(
                            out=out.ap()[blk], in_=az[:, 0, :, :])

    nc.finalize()
    return nc


def _perm_for_mask(mask_b):
    # stable sort, unmasked (False) first
    return np.argsort(mask_b, kind="stable")


def _pack_aln(aln_b, perm):
    # aln_p[tg, p, c, g*64+k] = aln[tg*GT+g, perm[c*128+p], k], bf16
    a16 = np.asarray(aln_b, np.float32).astype(BF)[:, perm, :]
    a16 = a16.reshape(TG, GT, NJ, 128, A).transpose(0, 3, 2, 1, 4)
    return np.ascontiguousarray(a16).reshape(TG, 128, NJ, GT * A)


def _unpack_out(o, perm):
    # inverse of _pack_aln, bf16 -> fp32 [T, S, A]
    o = np.asarray(o).reshape(TG, 128, NJ, GT, A).transpose(0, 3, 2, 1, 4)
    o = np.ascontiguousarray(o).reshape(T, S, A).astype(np.float32)
    res = np.empty_like(o)
    res[:, perm, :] = o
    return res


def _prep_in_maps(signal, bases, aln, signal_mask,
                  sig_norm_g, sig_norm_b, bases_norm_g, bases_norm_b,
                  sig_W, bases_W, out_W, out_b):
    signal = np.asarray(signal, np.float32)
    bases = np.asarray(bases, np.float32)
    aln = np.asarray(aln, np.float32)
    mask = np.asarray(signal_mask)
    A1 = np.ascontiguousarray(
        (np.asarray(sig_W, np.float32) * np.asarray(sig_norm_g, np.float32)).T
    ).astype(BF)
    c1 = (np.asarray(sig_W, np.float32) @ np.asarray(sig_norm_b, np.float32))[
        None].astype(BF)
    A2 = np.ascontiguousarray(
        (np.asarray(bases_W, np.float32) * np.asarray(bases_norm_g, np.float32)).T
    ).astype(BF)
    c2 = (np.asarray(bases_W, np.float32) @ np.asarray(bases_norm_b, np.float32))[
        None].astype(BF)
    WtT = np.ascontiguousarray(np.asarray(out_W, np.float32).T)
    outbrep = np.ascontiguousarray(
        np.tile(np.asarray(out_b, np.float32), T)[None]
    ).astype(BF)

    perms = [_perm_for_mask(mask[b]) for b in range(B)]
    eu = max(
        int(np.ceil((~mask[b]).sum() / 128.0)) for b in range(B))
    eu = max(eu, 1)

    in_maps = []
    for b in range(B):
        perm = perms[b]
        mf_s = (1.0 - mask[b].astype(np.float32))[perm]  # sorted; 1 = unmasked
        sig_dev = signal[b][perm[:128 * eu]].reshape(eu, 128, E)
        sig_dev = np.ascontiguousarray(sig_dev.transpose(1, 0, 2))
        in_maps.append({
            "signal": sig_dev,
            "bases": np.ascontiguousarray(bases[b]),
            "aln": _pack_aln(aln[b], perm),
            "maskT": np.ascontiguousarray(
                mf_s[:128 * eu].reshape(eu, 128).T),
            "mrow": np.ascontiguousarray(
                mf_s[:128 * eu][None]).astype(BF),
            "A1": A1, "c1": np.ascontiguousarray(c1),
            "A2": A2, "c2": np.ascontiguousarray(c2),
            "WtT": WtT, "outbrep": outbrep,
        })
    return in_maps, perms, eu


def _gather_out(res, perms):
    return np.stack(
        [_unpack_out(res.results[i]["out"], perms[i]) for i in range(B)],
        axis=0)


def _run(inputs, **kw):
    in_maps, perms, eu = _prep_in_maps(**inputs)
    nc = build_nc(eu)
    res = run_bass_kernel_spmd(nc, in_maps, core_ids=list(range(B)), **kw)
    return _gather_out(res, perms), res


def kernel(**inputs) -> np.ndarray:
    out, _ = _run(inputs)
    return out


# revision 31
# speedup vs baseline: 1.0504x; 1.0311x over previous
"""AlignmentBlock kernel for 8 TRN2 NeuronCores (data-parallel over B).

Math (per batch b, one core per batch):
  s_hat[s,a] = (LN(signal[s]) * g1 + b1) @ sig_W.T, zeroed where signal_mask
  b_hat[t,a] = (LN(bases[t]) * g2 + b2) @ bases_W.T
  out[t,s,k] = aln[t,s,k] + gelu( sum_a b_hat[t,a]*s_hat[s,a]*out_W[k,a] + out_b[k] )

Key tricks (measured 148.5us -> ~81us on HW):
- The [B,T,S,A] intermediate is never materialized: for each t the
  projection collapses to s_hat @ (out_W.T * b_hat[t]) — a [65,128]^T x
  [65, 8t*64] matmul per 128-column chunk of s_hat (row 65 of the
  stationary is ones, row 65 of the moving operand is out_b).
- bf16 aln I/O: the host packs aln[b] to bf16 in the exact SBUF tile
  layout and unpacks the bf16 result to fp32 after the run.  Halves HBM
  traffic (2 x 12.6MB instead of 2 x 25.2MB per core); every aln DMA is
  a fully contiguous 1-2MB transfer.  The layout is c-outer
  (aln_p[tg, p, c, g*64+k]) so every epilogue op hits contiguous
  512-element runs per partition.
- Mask sort: the s axis is freely permutable (the host controls the
  signal row order, the aln packing and the unpacking), so each core's
  s positions are sorted unmasked-first.  For a fully-masked 128-column
  chunk the projection input is exactly out_b, so out = aln +
  gelu(out_b) — one broadcast bf16 add, no matmul / no gelu.  Only
  eu = max_b ceil(unmasked_b/128) chunks (5 of 8 for ~50% masks) take
  the matmul path; the kernel graph is built per call with that eu.
- Queue split: aln loads own the sync HWDGE ring (params first, they're
  small), stores own the scalar HWDGE ring; with both streams the kernel
  sits at the shared HBM-stack roofline (~26MB/core at ~370GB/s).
- Epilogue fused pairwise: two matmuls fill a 2-bank PSUM tile, then ONE
  gelu (ScalarE, f32 -> bf16) and ONE contiguous bf16 add (VectorE, 2x
  rate) cover both banks.  wfull (out_W.T * b_hat) is built per group
  just-in-time on VectorE so it hides in epilogue slack instead of
  delaying the signal-branch preamble.  GpSimd runs no bulk tensor ops —
  its SBUF traffic stalls concurrent DVE work 3-6x.

Matmul operands are bf16; accumulation stays fp32 in PSUM. LayerNorm
affine and all parameter reshapes/transposes are folded on the host.
"""

import numpy as np
import ml_dtypes

import concourse.bass as bass
import concourse.tile as tile
from concourse import bacc, mybir
from concourse.bass_utils import run_bass_kernel_spmd
from concourse.masks import make_identity

F32 = mybir.dt.float32
BF16 = mybir.dt.bfloat16
AF = mybir.ActivationFunctionType
ALU = mybir.AluOpType

B, T, S, E, A = 8, 96, 1024, 256, 64
LN_EPS = 1e-5
GT = 8           # t's per packed aln block (1MB bf16)
TG = T // GT     # 12 blocks
NJ = S // 128    # 8 s-chunks of 128
BF = ml_dtypes.bfloat16

# main-loop groups: (block index, #blocks fused into one az tile);
# small groups first (early first store) and last (short drain tail)
GROUPS = [(0, 1), (1, 1), (2, 2), (4, 2), (6, 2), (8, 2), (10, 1), (11, 1)]


def build_nc(eu=NJ):
    nc = bacc.Bacc(target_bir_lowering=False)

    sig = nc.declare_dram_parameter("signal", [128, eu, E], F32, isOutput=False)
    bas = nc.declare_dram_parameter("bases", [T, E], F32, isOutput=False)
    aln = nc.declare_dram_parameter("aln", [TG, 128, NJ, GT * A], BF16,
                                    isOutput=False)
    mskT = nc.declare_dram_parameter("maskT", [128, eu], F32, isOutput=False)
    mrow = nc.declare_dram_parameter("mrow", [1, eu * 128], BF16, isOutput=False)
    A1 = nc.declare_dram_parameter("A1", [E, A], BF16, isOutput=False)
    c1 = nc.declare_dram_parameter("c1", [1, A], BF16, isOutput=False)
    A2 = nc.declare_dram_parameter("A2", [E, A], BF16, isOutput=False)
    c2 = nc.declare_dram_parameter("c2", [1, A], BF16, isOutput=False)
    WtT = nc.declare_dram_parameter("WtT", [A, A], F32, isOutput=False)
    outbrep = nc.declare_dram_parameter("outbrep", [1, T * A], BF16,
                                        isOutput=False)
    out = nc.declare_dram_parameter("out", [TG, 128, NJ, GT * A], BF16,
                                    isOutput=True)

    aln2 = aln.ap().rearrange("(u v) p c x -> u p v c x", v=2)
    out2 = out.ap().rearrange("(u v) p c x -> u p v c x", v=2)

    with tile.TileContext(nc) as tc:
        with (
            tc.tile_pool(name="singles", bufs=1) as singles,
            tc.tile_pool(name="alnp2", bufs=4) as alnp2,
            tc.tile_pool(name="alnp1", bufs=4) as alnp1,
        ):
            # ---------- constants / params (sync ring, ahead of aln) ----------
            ident = singles.tile([128, 128], F32)
            make_identity(nc, ident)
            eps_t = singles.tile([128, 1], F32)
            nc.vector.memset(eps_t, LN_EPS)
            ones_row = singles.tile([1, 128], BF16)
            nc.vector.memset(ones_row, 1.0)

            bas_sb = singles.tile([T, E], F32)
            nc.sync.dma_start(out=bas_sb, in_=bas.ap())
            m_sb = singles.tile([128, eu], F32)
            nc.sync.dma_start(out=m_sb, in_=mskT.ap())
            m_row = singles.tile([1, eu * 128], BF16)
            nc.sync.dma_start(out=m_row, in_=mrow.ap())
            A1_sb = singles.tile([128, 2, A], BF16)
            nc.sync.dma_start(
                out=A1_sb, in_=A1.ap().rearrange("(h e) a -> e h a", e=128)
            )
            A2_sb = singles.tile([128, 2, A], BF16)
            nc.sync.dma_start(
                out=A2_sb, in_=A2.ap().rearrange("(h e) a -> e h a", e=128)
            )
            c1_sb = singles.tile([1, A], BF16)
            nc.sync.dma_start(out=c1_sb, in_=c1.ap())
            c2_sb = singles.tile([1, A], BF16)
            nc.sync.dma_start(out=c2_sb, in_=c2.ap())
            WtT_sb = singles.tile([A, A], F32)
            nc.sync.dma_start(out=WtT_sb, in_=WtT.ap())
            wfull = singles.tile([A + 1, T, A], BF16)
            nc.sync.dma_start(
                out=wfull[A:A + 1, :, :],
                in_=outbrep.ap().rearrange("x (t k) -> x t k", t=T),
            )
            sig_sb = singles.tile([128, eu, E], F32)
            nc.sync.dma_start(out=sig_sb, in_=sig.ap())

            with (
                tc.tile_pool(name="pre", bufs=2) as pre,
                tc.tile_pool(name="psum_pre", bufs=2, space="PSUM") as psum_pre,
            ):
                # ---------- bases branch: LN + project -> bhT [64, 96] ----------
                bst = pre.tile([T, 6], F32, tag="pp_small")
                nc.vector.bn_stats(bst, bas_sb)
                bmv = pre.tile([T, 2], F32, tag="pp_small")
                nc.vector.bn_aggr(bmv, bst)
                brs = pre.tile([T, 1], F32, tag="pp_small")
                nc.scalar.activation(brs, bmv[:, 1:2], AF.Sqrt, bias=eps_t[0:T])
                nc.vector.reciprocal(brs, brs)
                xb = pre.tile([T, E], F32)
                nc.vector.tensor_scalar(
                    out=xb, in0=bas_sb, scalar1=bmv[:, 0:1], scalar2=brs,
                    op0=ALU.subtract, op1=ALU.mult,
                )
                xbT = pre.tile([128, 2, T], BF16)
                for h in range(2):
                    ptr = psum_pre.tile([128, T], F32, tag="pp_psum")
                    nc.tensor.transpose(
                        ptr, xb[:, h * 128:(h + 1) * 128], ident[0:T, 0:T]
                    )
                    nc.scalar.copy(xbT[:, h, :], ptr)
                bh_ps = psum_pre.tile([A, T], F32, tag="pp_psum")
                nc.tensor.matmul(bh_ps, A2_sb[:, 0, :], xbT[:, 0, :],
                                 start=True, stop=False)
                nc.tensor.matmul(bh_ps, A2_sb[:, 1, :], xbT[:, 1, :],
                                 start=False, stop=False)
                nc.tensor.matmul(bh_ps, c2_sb, ones_row[:, 0:T],
                                 start=False, stop=True)
                bhT = singles.tile([A, T], F32)
                nc.vector.tensor_copy(bhT, bh_ps)

                # ---------- signal branch: LN (masked) + transpose ----------
                # high_priority: this chain gates the first matmul, gelu,
                # add and store; without it the scheduler front-loads the
                # wfull pieces on VectorE and delays the store stream ~10us
                with tc.high_priority():
                    xnT = singles.tile([128, 2, eu * 128], BF16)
                    st8 = pre.tile([128, eu, 6], F32, tag="pp_small")
                    for j in range(eu):
                        nc.vector.bn_stats(st8[:, j, :], sig_sb[:, j, :])
                    mv8 = pre.tile([128, eu, 2], F32, tag="pp_small")
                    for j in range(eu):
                        nc.vector.bn_aggr(mv8[:, j, :], st8[:, j, :])
                    rsm8 = pre.tile([128, eu], F32, tag="pp_small")
                    nc.scalar.activation(rsm8, mv8[:, :, 1], AF.Sqrt,
                                         bias=eps_t)
                    nc.vector.reciprocal(rsm8, rsm8)
                    nc.vector.tensor_mul(rsm8, rsm8, m_sb)
                    for j in range(eu):
                        xn = pre.tile([128, E], F32)
                        nc.vector.tensor_scalar(
                            out=xn, in0=sig_sb[:, j, :],
                            scalar1=mv8[:, j, 0:1], scalar2=rsm8[:, j:j + 1],
                            op0=ALU.subtract, op1=ALU.mult,
                        )
                        ptr = psum_pre.tile([128, 256], F32, tag="pp_psum")
                        for h in range(2):
                            nc.tensor.transpose(
                                ptr[:, h * 128:(h + 1) * 128],
                                xn[:, h * 128:(h + 1) * 128], ident)
                        nc.scalar.copy(
                            xnT[:, :, j * 128:(j + 1) * 128],
                            ptr.rearrange("p (h q) -> p h q", h=2))

                    # ------- project signal -> shp [65, eu*128] bf16 -------
                    # column u = C*128 + p <-> sorted s position; row 64 = ones
                    shp = singles.tile([A + 1, eu * 128], BF16)
                    for j in range(eu):
                        cs = slice(j * 128, (j + 1) * 128)
                        pp = psum_pre.tile([A, 128], F32, tag="pp_psum")
                        nc.tensor.matmul(
                            pp, A1_sb[:, 0, :], xnT[:, 0, cs],
                            start=True, stop=False)
                        nc.tensor.matmul(
                            pp, A1_sb[:, 1, :], xnT[:, 1, cs],
                            start=False, stop=False)
                        nc.tensor.matmul(
                            pp, c1_sb, m_row[:, cs],
                            start=False, stop=True)
                        nc.scalar.copy(shp[0:A, cs], pp)
                    nc.vector.memset(shp[A:A + 1, :], 1.0)

                # ---------- gelu(out_b) broadcast tile for masked chunks -------
                # (emitted after the shp chain so its ACT ops don't delay shp;
                # only the cheap adds consume it)
                if eu < NJ:
                    g1 = singles.tile([1, A], BF16)
                    nc.scalar.activation(g1, wfull[A:A + 1, 0, :], AF.Gelu)
                    gps = psum_pre.tile([128, A], F32, tag="pp_psum")
                    nc.tensor.matmul(gps, ones_row, g1, start=True, stop=True)
                    gcbr = singles.tile([128, GT, A], BF16)
                    for i in range(GT):
                        nc.scalar.copy(gcbr[:, i, :], gps)

            # ---------- main loop ----------
            with (
                tc.tile_pool(name="zp", bufs=8) as zp,
                tc.tile_pool(name="psum_main", bufs=4, space="PSUM") as psum_main,
            ):
                for blk, nb in GROUPS:
                    if nb == 2:
                        az = alnp2.tile([128, 2, NJ, GT * A], BF16)
                        nc.sync.dma_start(out=az, in_=aln2[blk // 2])
                    else:
                        az = alnp1.tile([128, 1, NJ, GT * A], BF16)
                        nc.sync.dma_start(
                            out=az[:, 0, :, :], in_=aln.ap()[blk])
                    # wfull piece for this group's t-range (rows 0..63:
                    # out_W.T * b_hat[t]); built just-in-time on VectorE so
                    # it hides in the epilogue slack instead of delaying shp
                    tw = nb * GT
                    t0w = blk * GT
                    WtT_bc = bass.AP(
                        tensor=WtT_sb.tensor, offset=WtT_sb.offset,
                        ap=[WtT_sb.ap[0], [0, tw], WtT_sb.ap[1]],
                    )
                    bq = bhT[:, t0w:t0w + tw]
                    bhT_bc = bass.AP(
                        tensor=bq.tensor, offset=bq.offset,
                        ap=[bq.ap[0], bq.ap[1], [0, A]],
                    )
                    nc.vector.tensor_tensor(
                        wfull[0:A, t0w:t0w + tw, :], WtT_bc, bhT_bc, ALU.mult)
                    for c in range(eu):
                        ps = psum_main.tile([128, 2, GT, A], F32)
                        for h in range(nb):
                            t0 = (blk + h) * GT
                            nc.tensor.matmul(
                                ps[:, h, :, :],
                                shp[:, c * 128:(c + 1) * 128],
                                wfull[:, t0:t0 + GT, :],
                                start=True, stop=True,
                            )
                        zc = zp.tile([128, 2, GT, A], BF16)
                        nc.scalar.activation(
                            zc[:, 0:nb, :, :], ps[:, 0:nb, :, :], AF.Gelu)
                        zcv = zc.rearrange("p v g k -> p v (g k)")
                        sl = az[:, 0:nb, c, :]
                        nc.vector.tensor_add(sl, sl, zcv[:, 0:nb, :])
                    if eu < NJ:
                        slm = az[:, 0:nb, eu:NJ, :]
                        gb = bass.AP(
                            tensor=gcbr.tensor, offset=gcbr.offset,
                            ap=[gcbr.ap[0], [0, nb], [0, NJ - eu], [1, GT * A]],
                        )
                        nc.vector.tensor_tensor(slm, slm, gb, ALU.add)
                    if nb == 2:
                        nc.scalar.dma_start(out=out2[blk // 2], in_=az)
                    else:
                        nc.scalar.dma_start(
                            out=out.ap()[blk], in_=az[:, 0, :, :])

    nc.finalize()
    return nc


def _perm_for_mask(mask_b):
    # stable sort, unmasked (False) first
    return np.argsort(mask_b, kind="stable")


def _pack_aln(aln_b, perm):
    # aln_p[tg, p, c, g*64+k] = aln[tg*GT+g, perm[c*128+p], k], bf16
    a16 = np.asarray(aln_b, np.float32).astype(BF)[:, perm, :]
    a16 = a16.reshape(TG, GT, NJ, 128, A).transpose(0, 3, 2, 1, 4)
    return np.ascontiguousarray(a16).reshape(TG, 128, NJ, GT * A)


def _unpack_out(o, perm):
    # inverse of _pack_aln, bf16 -> fp32 [T, S, A]
    o = np.asarray(o).reshape(TG, 128, NJ, GT, A).transpose(0, 3, 2, 1, 4)
    o = np.ascontiguousarray(o).reshape(T, S, A).astype(np.float32)
    res = np.empty_like(o)
    res[:, perm, :] = o
    return res


def _prep_in_maps(signal, bases, aln, signal_mask,
                  sig_norm_g, sig_norm_b, bases_norm_g, bases_norm_b,
                  sig_W, bases_W, out_W, out_b):
    signal = np.asarray(signal, np.float32)
    bases = np.asarray(bases, np.float32)
    aln = np.asarray(aln, np.float32)
    mask = np.asarray(signal_mask)
    A1 = np.ascontiguousarray(
        (np.asarray(sig_W, np.float32) * np.asarray(sig_norm_g, np.float32)).T
    ).astype(BF)
    c1 = (np.asarray(sig_W, np.float32) @ np.asarray(sig_norm_b, np.float32))[
        None].astype(BF)
    A2 = np.ascontiguousarray(
        (np.asarray(bases_W, np.float32) * np.asarray(bases_norm_g, np.float32)).T
    ).astype(BF)
    c2 = (np.asarray(bases_W, np.float32) @ np.asarray(bases_norm_b, np.float32))[
        None].astype(BF)
    WtT = np.ascontiguousarray(np.asarray(out_W, np.float32).T)
    outbrep = np.ascontiguousarray(
        np.tile(np.asarray(out_b, np.float32), T)[None]
    ).astype(BF)

    perms = [_perm_for_mask(mask[b]) for b in range(B)]
    eu = max(
        int(np.ceil((~mask[b]).sum() / 128.0)) for b in range(B))
    eu = max(eu, 1)

    in_maps = []
    for b in range(B):
        perm = perms[b]
        mf_s = (1.0 - mask[b].astype(np.float32))[perm]  # sorted; 1 = unmasked
        sig_dev = signal[b][perm[:128 * eu]].reshape(eu, 128, E)
        sig_dev = np.ascontiguousarray(sig_dev.transpose(1, 0, 2))
        in_maps.append({
            "signal": sig_dev,
            "bases": np.ascontiguousarray(bases[b]),
            "aln": _pack_aln(aln[b], perm),
            "maskT": np.ascontiguousarray(
                mf_s[:128 * eu].reshape(eu, 128).T),
            "mrow": np.ascontiguousarray(
                mf_s[:128 * eu][None]).astype(BF),
            "A1": A1, "c1": np.ascontiguousarray(c1),
            "A2": A2, "c2": np.ascontiguousarray(c2),
            "WtT": WtT, "outbrep": outbrep,
        })
    return in_maps, perms, eu


def _gather_out(res, perms):
    return np.stack(
        [_unpack_out(res.results[i]["out"], perms[i]) for i in range(B)],
        axis=0)


def _run(inputs, **kw):
    in_maps, perms, eu = _prep_in_maps(**inputs)
    nc = build_nc(eu)
    res = run_bass_kernel_spmd(nc, in_maps, core_ids=list(range(B)), **kw)
    return _gather_out(res, perms), res


def kernel(**inputs) -> np.ndarray:
    out, _ = _run(inputs)
    return out
